# revision 26
# baseline (speedup 1.0000x reference)
"""Trainium2 Bass kernel for nn_BoxAwareAttention: full attention block
(QKV proj + bias, RoPE, scaled-dot-product attention with softmax, out proj).

Sharding over 8 NeuronCores: data-parallel over batch (2) x tensor-parallel
over heads (16 -> 4 per core).  Core c handles batch c//4, heads 4*(c%4)..+4.
Each core computes its partial projection output (contraction over its 256
channels); host sums the 4 partials per batch.

Device-side layout choices:
  - All matmul inputs bf16 (fp32 PSUM accumulation).  Host pre-transposes
    and casts: x^T in 8 K-chunks of 128; weight slices pre-transposed per
    core.  q/k biases added on ScalarE (per-partition activation bias), v
    bias via a pre-broadcast table in the DVE evacuation add.
  - q,k produced transposed [d, L] (2 heads per 128-partition M-tile); RoPE
    applied in that layout on DVE (rotate-half = partition-shifted reads
    from PSUM — legal because the PSUM operand is exempt from the equal-
    base-partition rule; sign folded into the sin table).  v produced
    natural [L, d] with an appended ones column so the softmax denominator
    falls out of the P@V matmul as an extra output row.
  - Scores computed transposed S^T = K^T-tile.T @ Q^T per 128-k-tile into
    [128, 1024] 2-bank PSUM tiles; one exp ACTIVATE per k-tile (scale=1/8
    fused, bf16 out) amortizes ScalarE's 352-cycle fixed cost; P@V
    accumulates v_aug-stationary over 16 k-tiles into [65, 512] PSUM tiles;
    normalization = fast-reciprocal (DVE) + partition_broadcast (GpSimd,
    base-0 APs only — it mishandles nonzero AP base partitions) + multiply.
  - Out-proj contracts the core's 256 channels in 2 K=128 chunks from the
    stacked oT tiles; PSUM evacuated on DVE, DMA per 128-row tile.
"""

import os
import sys

for _p in ("/opt/trn_rl_repo", "/root/.axon_site/_ro/trn_rl_repo"):
    if os.path.isdir(_p) and _p not in sys.path:
        sys.path.insert(0, _p)

import numpy as np
import ml_dtypes

import concourse.bass as bass
import concourse.mybir as mybir
import concourse.tile as tile
from concourse import bacc
from concourse import bass_utils

BF16 = ml_dtypes.bfloat16
N_CORES = 8
B, L_FULL, C, H, D = 2, 2048, 1024, 16, 64
H_LOC = 4              # heads per core
M_LOC = H_LOC * D      # 256 output channels per core
KCH = 8                # qkv contraction chunks (1024 = 8*128); biases added on DVE
F32 = mybir.dt.float32
BF = mybir.dt.bfloat16
ADD = mybir.AluOpType.add
MULT = mybir.AluOpType.mult


def build_program(L=L_FULL, num_devices=N_CORES, debug_taps=False):
    """Build the per-core Bass program (SPMD: same program, per-core data)."""
    NB = min(1024, L)      # q/L block width (PSUM tile free size)
    nNB = L // NB          # number of blocks
    nKT = L // 128         # attention k-tiles / v L-tiles
    nMT = 2                # q/k M-tiles (2 heads of 64 each)
    nQT = L // 128         # proj q-tiles
    PW = min(512, NB)      # PV/norm sub-block width
    SPL = [bass.ds(s, PW) for s in range(0, NB, PW)]

    nc = bacc.Bacc("TRN2", target_bir_lowering=False, debug=False,
                   num_devices=num_devices)
    taps = {}
    if debug_taps:
        taps = {
            "dbg_qT0": nc.dram_tensor("dbg_qT0", [128, L], BF, kind="ExternalOutput").ap(),
            "dbg_kT0": nc.dram_tensor("dbg_kT0", [128, L], BF, kind="ExternalOutput").ap(),
            "dbg_vaug": nc.dram_tensor("dbg_vaug", [128, nKT, H_LOC, 65], BF, kind="ExternalOutput").ap(),
            "dbg_oT0": nc.dram_tensor("dbg_oT0", [128, L], BF, kind="ExternalOutput").ap(),
        }

    xa_d = nc.dram_tensor("xa", [128, KCH, L], BF, kind="ExternalInput").ap()
    wq_d = nc.dram_tensor("wq", [128, KCH, M_LOC], BF, kind="ExternalInput").ap()
    wk_d = nc.dram_tensor("wk", [128, KCH, M_LOC], BF, kind="ExternalInput").ap()
    wv_d = nc.dram_tensor("wv", [128, KCH, M_LOC], BF, kind="ExternalInput").ap()
    wp_d = nc.dram_tensor("wp", [128, 2, C], BF, kind="ExternalInput").ap()
    bqk_d = nc.dram_tensor("bqk", [128, 4], F32, kind="ExternalInput").ap()
    bv_d = nc.dram_tensor("bv", [128, M_LOC], F32, kind="ExternalInput").ap()
    cos_d = nc.dram_tensor("cosT", [128, L], F32, kind="ExternalInput").ap()
    sinx_d = nc.dram_tensor("sinX", [128, L], F32, kind="ExternalInput").ap()
    o_d = nc.dram_tensor("o", [nQT, 128, C], F32, kind="ExternalOutput").ap()

    with tile.TileContext(nc) as tc:
        with (
            tc.tile_pool(name="const", bufs=1) as cpool,
            tc.tile_pool(name="rope", bufs=2) as rpool,
            tc.tile_pool(name="pt", bufs=17) as ptpool,
            tc.tile_pool(name="norm", bufs=2) as npool,
            tc.tile_pool(name="outs", bufs=2) as opool,
            tc.tile_pool(name="ps_big", bufs=2, space="PSUM") as ps_big,
            tc.tile_pool(name="ps_proj", bufs=1, space="PSUM") as ps_proj,
            tc.tile_pool(name="ps_o", bufs=2, space="PSUM") as ps_o,
        ):
            xa = [cpool.tile([128, L], BF, tag=f"xa{c}", name=f"xa{c}")
                  for c in range(KCH)]
            wq = cpool.tile([128, KCH, M_LOC], BF, tag="wq")
            wk = cpool.tile([128, KCH, M_LOC], BF, tag="wk")
            wv = cpool.tile([128, KCH, M_LOC], BF, tag="wv")
            wp = cpool.tile([128, 2, C], BF, tag="wp")
            bqk = cpool.tile([128, 4], F32, tag="bqk")
            bv = cpool.tile([128, M_LOC], F32, tag="bv")
            cos_s = cpool.tile([128, L], F32, tag="cos")
            sinx_s = cpool.tile([128, L], F32, tag="sinx")
            qT = [cpool.tile([128, L], BF, tag=f"qT{m}", name=f"qT{m}") for m in range(nMT)]
            kT = [cpool.tile([128, L], BF, tag=f"kT{m}", name=f"kT{m}") for m in range(nMT)]
            oT = [cpool.tile([128, L], BF, tag=f"oT{m}", name=f"oT{m}") for m in range(nMT)]
            v_aug = cpool.tile([128, nKT, H_LOC, 65], BF, tag="vaug")

            nc.sync.dma_start(wk[:, 0, :], wk_d[:, 0, :])
            nc.sync.dma_start(xa[0][:], xa_d[:, 0, :])
            nc.sync.dma_start(wk[:, 1:, :], wk_d[:, 1:, :])
            for c in range(1, KCH):
                nc.sync.dma_start(xa[c][:], xa_d[:, c, :])
            nc.sync.dma_start(bqk[:], bqk_d[:])
            nc.sync.dma_start(cos_s[:], cos_d[:])
            nc.sync.dma_start(sinx_s[:], sinx_d[:])
            nc.sync.dma_start(wv[:], wv_d[:])
            nc.sync.dma_start(bv[:], bv_d[:])
            nc.sync.dma_start(wq[:], wq_d[:])
            nc.sync.dma_start(wp[:], wp_d[:])
            nc.vector.memset(v_aug[:, :, :, 64:65], 1.0)

            # ---- helper: one q/k projection tile + bias + RoPE ----
            def qk_tile(w_s, dstT, mt, lb):
                ls = bass.ds(lb * NB, NB)
                ti = 0 if w_s is wq else 1
                pq = ps_big.tile([128, NB], F32, tag="big", name=f"pq{ti}{mt}{lb}")
                for cc in range(KCH):
                    for sp in SPL:
                        nc.tensor.matmul(
                            pq[:, sp],
                            w_s[:, cc, bass.ts(mt, 128)],
                            xa[cc][:, ls][:, sp],
                            start=(cc == 0), stop=(cc == KCH - 1))
                nc.scalar.activation(
                    pq[:], pq[:],
                    mybir.ActivationFunctionType.Identity,
                    bias=bqk[:, ti * 2 + mt:ti * 2 + mt + 1])
                rot = rpool.tile([128, NB], F32, tag="rot", name=f"rot{ti}{mt}{lb}")
                for do, so in ((0, 32), (32, 0), (64, 96), (96, 64)):
                    nc.vector.tensor_tensor(
                        rot[do:do + 32, :], pq[so:so + 32, :],
                        sinx_s[do:do + 32, ls], MULT)
                tcos = rpool.tile([128, NB], F32, tag="tcos", name=f"tcos{ti}{mt}{lb}")
                nc.vector.tensor_tensor(tcos[:], pq[:], cos_s[:, ls], MULT)
                nc.vector.tensor_tensor(dstT[mt][:, ls], tcos[:], rot[:], ADD)

            # ---- K projection (needed in full before any attention) ----
            for lb in range(nNB):
                for mt in range(nMT):
                    qk_tile(wk, kT, mt, lb)

            # ---- V natural [L, d] with ones column; bias added in evac ----
            for lt in range(nKT):
                pv = ps_big.tile([128, M_LOC], F32, tag="big")
                for cc in range(KCH):
                    nc.tensor.matmul(
                        pv[:], xa[cc][:, bass.ts(lt, 128)], wv[:, cc, :],
                        start=(cc == 0), stop=(cc == KCH - 1))
                nc.vector.tensor_tensor(
                    v_aug[:, lt, :, 0:64],
                    pv[:].rearrange("p (h d) -> p h d", h=H_LOC),
                    bv[:].rearrange("p (h d) -> p h d", h=H_LOC), ADD)

            # ---- Q projection ----
            for lb in range(nNB):
                for mt in range(nMT):
                    qk_tile(wq, qT, mt, lb)

            # ---- per q-block: attention, partial proj ----
            for qb in range(nNB):
                qs = bass.ds(qb * NB, NB)
                for h in range(H_LOC):
                    mt, hh = divmod(h, 2)
                    pr = slice(64 * hh, 64 * hh + 64)
                    pts = []
                    for kt in range(nKT):
                        st = ps_big.tile([128, NB], F32, tag="big")
                        for sp in SPL:
                            nc.tensor.matmul(
                                st[:, sp], kT[mt][pr, bass.ts(kt, 128)],
                                qT[mt][pr, qs][:, sp],
                                start=True, stop=True)
                        pt = ptpool.tile([128, NB], BF, tag="pt")
                        nc.scalar.activation(
                            pt[:], st[:], mybir.ActivationFunctionType.Exp,
                            scale=float(D) ** -0.5)
                        pts.append(pt)
                    pos = [ps_o.tile([65, PW], F32, tag="po", name=f"po{h}{si}")
                           for si in range(len(SPL))]
                    for kt in range(nKT):
                        for si, sp in enumerate(SPL):
                            nc.tensor.matmul(
                                pos[si][:], v_aug[:, kt, h, :], pts[kt][:, sp],
                                start=(kt == 0), stop=(kt == nKT - 1))
                    for si, sp in enumerate(SPL):
                        po = pos[si]
                        # softmax denominator: row 64 (ones column of v_aug)
                        dn = npool.tile([1, PW], F32, tag="dn")
                        nc.scalar.copy(dn[:], po[64:65, :])
                        rc = npool.tile([1, PW], F32, tag="rc")
                        nc.vector.reciprocal_approx_fast(rc[:], dn[:])
                        rb = npool.tile([64, PW], F32, tag="rb")
                        nc.gpsimd.partition_broadcast(rb[:], rc[:], channels=64)
                        nc.vector.tensor_tensor(
                            oT[mt][pr, qs][:, sp], po[0:64, :], rb[:], MULT)
                # ---- partial out-proj for this block ----
                for j in range(NB // 128):
                    qt = qb * (NB // 128) + j
                    pp = ps_proj.tile([128, C], F32, tag="pp")
                    for cc in range(2):
                        for nn in range(2):
                            nc.tensor.matmul(
                                pp[:, bass.ts(nn, C // 2)],
                                oT[cc][:, bass.ts(qt, 128)],
                                wp[:, cc, bass.ts(nn, C // 2)],
                                start=(cc == 0), stop=(cc == 1))
                    ost = opool.tile([128, C], F32, tag="ost")
                    if qt % 2 == 0:
                        nc.vector.tensor_copy(ost[:], pp[:])
                    else:
                        nc.scalar.copy(ost[:], pp[:])
                    nc.sync.dma_start(o_d[qt], ost[:])

            if debug_taps:
                nc.sync.dma_start(taps["dbg_qT0"][:], qT[0][:])
                nc.sync.dma_start(taps["dbg_kT0"][:], kT[0][:])
                nc.sync.dma_start(taps["dbg_vaug"][:], v_aug[:])
                nc.sync.dma_start(taps["dbg_oT0"][:], oT[0][:])

    nc.compile()
    return nc


_CACHE = {}


def _get_program(L=L_FULL):
    if L not in _CACHE:
        _CACHE[L] = build_program(L)
    return _CACHE[L]


def make_core_inputs(x, w_qkv, b_qkv, w_proj, cos, sin, L=L_FULL):
    """Host-side shard/transpose/pad/cast. Returns in_maps for the 8 cores."""
    x = np.asarray(x, np.float32)
    w_qkv = np.asarray(w_qkv, np.float32)
    b_qkv = np.asarray(b_qkv, np.float32)
    w_proj = np.asarray(w_proj, np.float32)
    cos = np.asarray(cos, np.float32)
    sin = np.asarray(sin, np.float32)

    # replicated rope tables: [128, L] (2 heads stacked), sign folded into sin
    cT = cos.T.astype(np.float32)                      # [64, L]
    sT = sin.T.astype(np.float32)
    cosT = np.concatenate([cT, cT], 0)                 # [128, L]
    sx = np.concatenate([-sT[0:32], sT[32:64]], 0)
    sinX = np.concatenate([sx, sx], 0)

    def pack_k(mat_t):
        # mat_t: [C, M] (already transposed) -> [128, KCH, M] bf16, chunk-major
        Cdim, M = mat_t.shape
        assert Cdim == KCH * 128
        return np.ascontiguousarray(
            mat_t.reshape(KCH, 128, M).transpose(1, 0, 2)).astype(BF16)

    in_maps = []
    for c in range(N_CORES):
        b, hg = divmod(c, 4)
        h0 = H_LOC * hg
        r = slice(h0 * D, (h0 + H_LOC) * D)            # head-channel rows
        # per-partition q/k biases for the [d, L] layout (cols: q-mt0,
        # q-mt1, k-mt0, k-mt1), and v bias broadcast across partitions
        bq, bk = b_qkv[r], b_qkv[C:][r]
        bqk = np.stack([bq[:128], bq[128:], bk[:128], bk[128:]], 1)
        bv = np.tile(b_qkv[2 * C:][r][None, :], (128, 1))
        in_maps.append({
            "xa": pack_k(x[b].T),
            "wq": pack_k(w_qkv[r].T),
            "wk": pack_k(w_qkv[C:][r].T),
            "wv": pack_k(w_qkv[2 * C:][r].T),
            "wp": np.ascontiguousarray(
                w_proj[:, r].T.reshape(2, 128, C).transpose(1, 0, 2)).astype(BF16),
            "bqk": np.ascontiguousarray(bqk, np.float32),
            "bv": np.ascontiguousarray(bv, np.float32),
            "cosT": cosT,
            "sinX": sinX,
        })
    return in_maps


def kernel(x, w_qkv, b_qkv, w_proj, cos, sin, mask=None, trace=False):
    nc = _get_program()
    in_maps = make_core_inputs(x, w_qkv, b_qkv, w_proj, cos, sin)
    res = bass_utils.run_bass_kernel_spmd(
        nc, in_maps, core_ids=list(range(N_CORES)), trace=trace)
    out = np.zeros((B, L_FULL, C), np.float32)
    for c in range(N_CORES):
        out[c // 4] += res.results[c]["o"].reshape(L_FULL, C)
    if trace:
        kernel.last_results = res
    return out
